# revision 11
# baseline (speedup 1.0000x reference)
"""GCN message-passing kernel for 8 TRN2 NeuronCores.

Reference computation (per (b, c) pair, all fp32):
    e1  = x @ W1^T + b1          [N, H]
    e2  = x @ W2^T + b2          [N, H]
    adj = relu(e1 @ e2^T)        [N, N]
    h   = adj @ x                [N, F]
    out = h @ W3^T + b3          [N, O]

Sharding: the 32 (b, c) pairs are split 4-per-core across 8 cores;
weights are replicated. Each core runs an identical Bass program fully
fused in SBUF/PSUM (the N x N adjacency never touches HBM).

Layout trick: partition p of SBUF holds rows [16p, 16p+16) of the pair
(a pure row permutation, applied consistently to both sides of every
contraction and undone by the output DMA), which makes every HBM
transfer contiguous 4KB per partition.
"""

import sys

for _p in ("/opt/trn_rl_repo",):
    if _p not in sys.path:
        sys.path.insert(0, _p)

import numpy as np

import concourse.bass as bass
import concourse.tile as tile
from concourse import bacc, mybir
from concourse.bass import ts
from concourse.masks import make_identity

B, C, N, F = 4, 8, 2048, 64
H = 64
O = 64
NCORES = 8
PAIRS = (B * C) // NCORES  # 4 (b,c) pairs per core
P = 128                    # SBUF partitions
TBLK = N // P              # 16 row-blocks per pair
CH = 512                   # moving-operand chunk (one PSUM bank of fp32)
NCH = N // CH              # 4 chunks per row
F32 = mybir.dt.float32
F32R = mybir.dt.float32r

AF = mybir.ActivationFunctionType


class _EngineAlternator:
    """Round-robin PSUM->SBUF copy/relu work across Scalar and Vector."""

    def __init__(self, nc):
        self.nc = nc
        self.i = 0

    def copy(self, out, in_):
        self.i += 1
        if self.i % 2:
            self.nc.scalar.copy(out, in_)
        else:
            self.nc.vector.tensor_copy(out, in_)

    def relu(self, out, in_):
        self.i += 1
        if self.i % 2:
            self.nc.scalar.activation(out, in_, AF.Relu)
        else:
            self.nc.vector.tensor_scalar_max(out, in_, 0.0)


def _emit(tc, x_d, w_d, b_d, out_d, reps=1):
    nc = tc.nc
    eng = _EngineAlternator(nc)

    import contextlib

    with contextlib.ExitStack() as ctx:
        consts = ctx.enter_context(tc.tile_pool(name="consts", bufs=1))
        xpool = ctx.enter_context(tc.tile_pool(name="xp", bufs=2))
        xtpool = ctx.enter_context(tc.tile_pool(name="xt", bufs=2))
        epool = ctx.enter_context(tc.tile_pool(name="ep", bufs=2))
        adjpool = ctx.enter_context(tc.tile_pool(name="adj", bufs=4))
        htpool = ctx.enter_context(tc.tile_pool(name="ht", bufs=2))
        opool = ctx.enter_context(tc.tile_pool(name="op", bufs=2))
        ps_small = ctx.enter_context(tc.tile_pool(name="pss", bufs=2, space="PSUM"))
        ps_adj = ctx.enter_context(tc.tile_pool(name="psa", bufs=2, space="PSUM"))
        ps_h = ctx.enter_context(tc.tile_pool(name="psh", bufs=4, space="PSUM"))

        ident = consts.tile([P, P], F32)
        make_identity(nc, ident[:])

        # Augmented transposed weights: wa[k] rows 0..F-1 = Wk^T, row F = bk,
        # so e = Wa^T @ [xT; ones] folds the bias into the matmul (K = F+1).
        # w1a/w2a feed fp32r matmuls, so every producer writes fp32r
        # (the BIR verifier requires fp32r operands to be pre-rounded).
        was = []
        for k in range(3):
            dt_k = F32 if k == 2 else F32R
            wraw = consts.tile([H, F], F32, tag="wraw")
            nc.sync.dma_start(wraw[:], w_d[k][:])
            brow = consts.tile([1, H], F32, tag="brow")
            nc.sync.dma_start(brow[:], b_d[k].unsqueeze(0))
            wa = consts.tile([F + 1, H], dt_k, tag=f"wa{k}")
            pw = ps_small.tile([F, H], F32, tag="pss")
            nc.tensor.transpose(pw[:], wraw[:], ident[0:H, 0:H])
            nc.vector.tensor_copy(wa[0:F, :], pw[:])
            nc.vector.tensor_copy(wa[F : F + 1, :], brow[:])
            was.append(wa)
        w1a, w2a, w3a = was

        # Persistent xT tile (fp32r): Pool memset can't write fp32r, so the
        # ones row is seeded once from an fp32 staging row; rows 0..F-1 are
        # overwritten per pair.
        ones_row = consts.tile([1, N], F32)
        nc.gpsimd.memset(ones_row[:], 1.0)
        xta = consts.tile([F + 1, N], F32R)
        nc.vector.tensor_copy(xta[F : F + 1, :], ones_row[:])

        for _rep in range(reps):
            for p in range(PAIRS):
                # ---- load x pair as [128, 16, 64]; partition q = rows 16q..16q+15
                x_sb = xpool.tile([P, TBLK, F], F32, tag="x_sb")
                nc.sync.dma_start(
                    x_sb[:], x_d[p].rearrange("(q t) f -> q t f", q=P)
                )
                # fp32r-rounded copy of x for the hT-accumulation stationaries
                x_sbr = xpool.tile([P, TBLK * F], F32R, tag="x_sbr")
                eng.copy(x_sbr[:], x_sb[:].rearrange("q t f -> q (t f)"))

                # ---- xT (augmented with ones row): column t*128+q <-> row 16q+t
                for c in range(NCH):
                    pt = ps_small.tile([F, CH], F32, tag="pss")
                    for j in range(CH // P):
                        t = c * (CH // P) + j
                        nc.tensor.transpose(pt[:, ts(j, P)], x_sb[:, t, :], ident[:])
                    eng.copy(xta[0:F, ts(c, CH)], pt[:])

                # ---- e1T / e2T = Wa^T @ xTa   (biases included via K=65)
                ets = []
                for wa, tag in ((w1a, "e1t"), (w2a, "e2t")):
                    et = epool.tile([H, N], F32R, tag=tag)
                    for c in range(NCH):
                        pe_ = ps_small.tile([H, CH], F32, tag="pss")
                        nc.tensor.matmul(
                            pe_[:],
                            wa[:],
                            xta[:, ts(c, CH)],
                            start=True,
                            stop=True,
                        )
                        eng.copy(et[:, ts(c, CH)], pe_[:])
                    ets.append(et)
                e1t, e2t = ets

                # ---- main fused loop over 16 m-blocks:
                #      adjT_mb = relu(e2t[:, mb]^T @ e1t)      [128, 2048]
                #      hT     += x_mb(stationary) @ adjT_mb    [64, 2048] PSUM
                ph = [
                    ps_h.tile([F, CH], F32, tag="ph", name=f"ph{c}")
                    for c in range(NCH)
                ]
                for mb in range(TBLK):
                    for c in range(NCH):
                        pa = ps_adj.tile([P, CH], F32, tag="pa")
                        nc.tensor.matmul(
                            pa[:],
                            e2t[:, ts(mb, P)],
                            e1t[:, ts(c, CH)],
                            start=True,
                            stop=True,
                        )
                        asb = adjpool.tile([P, CH], F32R, tag="asb")
                        eng.relu(asb[:], pa[:])
                        nc.tensor.matmul(
                            ph[c][:],
                            x_sbr[:, ts(mb, F)],
                            asb[:],
                            start=(mb == 0),
                            stop=(mb == TBLK - 1),
                        )

                # ---- hT -> SBUF (augmented with ones row for the b3 fold)
                hta = htpool.tile([F + 1, N], F32, tag="hta")
                nc.gpsimd.memset(hta[F : F + 1, :], 1.0)
                for c in range(NCH):
                    eng.copy(hta[0:F, ts(c, CH)], ph[c][:])

                # ---- out = hTa^T @ W3a  (b3 via K=65), packed 8 blocks/bank
                out_sb = opool.tile([P, TBLK * O], F32, tag="out_sb")
                for g in range(2):
                    po = ps_small.tile([P, CH], F32, tag="pss")
                    for j in range(TBLK // 2):
                        nb = g * (TBLK // 2) + j
                        nc.tensor.matmul(
                            po[:, ts(j, O)],
                            hta[:, ts(nb, P)],
                            w3a[:],
                            start=True,
                            stop=True,
                        )
                    eng.copy(out_sb[:, ts(g, CH)], po[:])
                nc.sync.dma_start(
                    out_d[p].rearrange("(q t) f -> q (t f)", q=P), out_sb[:]
                )


def build_program(reps=1):
    nc = bacc.Bacc("TRN2", target_bir_lowering=False, debug=False)
    x_d = nc.dram_tensor("x", [PAIRS, N, F], F32, kind="ExternalInput").ap()
    w_d = [
        nc.dram_tensor(f"w{k}", [H, F], F32, kind="ExternalInput").ap()
        for k in (1, 2, 3)
    ]
    b_d = [
        nc.dram_tensor(f"b{k}", [H], F32, kind="ExternalInput").ap()
        for k in (1, 2, 3)
    ]
    out_d = nc.dram_tensor("out", [PAIRS, N, O], F32, kind="ExternalOutput").ap()
    with tile.TileContext(nc) as tc:
        _emit(tc, x_d, w_d, b_d, out_d, reps=reps)
    nc.compile()
    return nc


def make_in_maps(x, W1, b1, W2, b2, W3, b3):
    xs = np.ascontiguousarray(np.asarray(x, np.float32).reshape(B * C, N, F))
    const = {
        "w1": np.ascontiguousarray(np.asarray(W1, np.float32)),
        "w2": np.ascontiguousarray(np.asarray(W2, np.float32)),
        "w3": np.ascontiguousarray(np.asarray(W3, np.float32)),
        "b1": np.ascontiguousarray(np.asarray(b1, np.float32)),
        "b2": np.ascontiguousarray(np.asarray(b2, np.float32)),
        "b3": np.ascontiguousarray(np.asarray(b3, np.float32)),
    }
    return [
        {"x": np.ascontiguousarray(xs[i * PAIRS : (i + 1) * PAIRS]), **const}
        for i in range(NCORES)
    ]


_NC_CACHE = {}


def kernel(x, W1, b1, W2, b2, W3, b3):
    from concourse.bass_utils import run_bass_kernel_spmd

    if "nc" not in _NC_CACHE:
        _NC_CACHE["nc"] = build_program()
    nc = _NC_CACHE["nc"]
    in_maps = make_in_maps(x, W1, b1, W2, b2, W3, b3)
    res = run_bass_kernel_spmd(nc, in_maps, list(range(NCORES))).results
    out = np.concatenate([res[i]["out"] for i in range(NCORES)], axis=0)
    return out.reshape(B, C, N, O)


# revision 13
# speedup vs baseline: 10.4225x; 10.4225x over previous
"""GCN message-passing kernel for 8 TRN2 NeuronCores.

Reference computation (per (b, c) pair, all fp32):
    e1  = x @ W1^T + b1          [N, H]
    e2  = x @ W2^T + b2          [N, H]
    adj = relu(e1 @ e2^T)        [N, N]
    h   = adj @ x                [N, F]
    out = h @ W3^T + b3          [N, O]

Sharding: the 32 (b, c) pairs are split 4-per-core across 8 cores;
weights are replicated. Each core runs an identical Bass program fully
fused in SBUF/PSUM (the N x N adjacency never touches HBM).

Layout trick: partition p of SBUF holds rows [16p, 16p+16) of the pair
(a pure row permutation, applied consistently to both sides of every
contraction and undone by the output DMA), which makes every HBM
transfer contiguous 4KB per partition.
"""

import sys

for _p in ("/opt/trn_rl_repo",):
    if _p not in sys.path:
        sys.path.insert(0, _p)

import numpy as np

import concourse.bass as bass
import concourse.tile as tile
from concourse import bacc, mybir
from concourse.bass import ts
from concourse.masks import make_identity

B, C, N, F = 4, 8, 2048, 64
H = 64
O = 64
NCORES = 8
PAIRS = (B * C) // NCORES  # 4 (b,c) pairs per core
P = 128                    # SBUF partitions
TBLK = N // P              # 16 row-blocks per pair
CH = 512                   # moving-operand chunk (one PSUM bank of fp32)
NCH = N // CH              # 4 chunks per row
F32 = mybir.dt.float32
F32R = mybir.dt.float32r

AF = mybir.ActivationFunctionType


class _EngineAlternator:
    """Round-robin PSUM->SBUF copy/relu work across Scalar and Vector."""

    def __init__(self, nc):
        self.nc = nc
        self.i = 0

    def copy(self, out, in_):
        self.i += 1
        if self.i % 2:
            self.nc.scalar.copy(out, in_)
        else:
            self.nc.vector.tensor_copy(out, in_)

    def relu(self, out, in_):
        self.i += 1
        if self.i % 2:
            self.nc.scalar.activation(out, in_, AF.Relu)
        else:
            self.nc.vector.tensor_scalar_max(out, in_, 0.0)


def _emit(tc, x_d, w_d, b_d, out_d, reps=1):
    nc = tc.nc
    eng = _EngineAlternator(nc)

    import contextlib

    with contextlib.ExitStack() as ctx:
        consts = ctx.enter_context(tc.tile_pool(name="consts", bufs=1))
        xpool = ctx.enter_context(tc.tile_pool(name="xp", bufs=2))
        xtpool = ctx.enter_context(tc.tile_pool(name="xt", bufs=2))
        epool = ctx.enter_context(tc.tile_pool(name="ep", bufs=2))
        adjpool = ctx.enter_context(tc.tile_pool(name="adj", bufs=4))
        htpool = ctx.enter_context(tc.tile_pool(name="ht", bufs=2))
        opool = ctx.enter_context(tc.tile_pool(name="op", bufs=2))
        ps_small = ctx.enter_context(tc.tile_pool(name="pss", bufs=2, space="PSUM"))
        ps_adj = ctx.enter_context(tc.tile_pool(name="psa", bufs=2, space="PSUM"))
        ps_h = ctx.enter_context(tc.tile_pool(name="psh", bufs=4, space="PSUM"))

        ident = consts.tile([P, P], F32)
        make_identity(nc, ident[:])

        # Augmented transposed weights: wa[k] rows 0..F-1 = Wk^T, row F = bk,
        # so e = Wa^T @ [xT; ones] folds the bias into the matmul (K = F+1).
        # w1a/w2a feed fp32r matmuls, so every producer writes fp32r
        # (the BIR verifier requires fp32r operands to be pre-rounded).
        was = []
        for k in range(3):
            dt_k = F32 if k == 2 else F32R
            wraw = consts.tile([H, F], F32, tag="wraw")
            nc.sync.dma_start(wraw[:], w_d[k][:])
            brow = consts.tile([1, H], F32, tag="brow")
            nc.sync.dma_start(brow[:], b_d[k].unsqueeze(0))
            wa = consts.tile([F + 1, H], dt_k, tag=f"wa{k}")
            pw = ps_small.tile([F, H], F32, tag="pss")
            nc.tensor.transpose(pw[:], wraw[:], ident[0:H, 0:H])
            nc.vector.tensor_copy(wa[0:F, :], pw[:])
            nc.vector.tensor_copy(wa[F : F + 1, :], brow[:])
            was.append(wa)
        w1a, w2a, w3a = was

        # Persistent xT tile (fp32r): Pool memset can't write fp32r, so the
        # ones row is seeded once from an fp32 staging row; rows 0..F-1 are
        # overwritten per pair.
        ones_row = consts.tile([1, N], F32)
        nc.gpsimd.memset(ones_row[:], 1.0)
        xta = consts.tile([F + 1, N], F32R)
        nc.vector.tensor_copy(xta[F : F + 1, :], ones_row[:])

        def body():
            for p in range(PAIRS):
                # ---- load x pair as [128, 16, 64]; partition q = rows 16q..16q+15
                x_sb = xpool.tile([P, TBLK, F], F32, tag="x_sb")
                nc.sync.dma_start(
                    x_sb[:], x_d[p].rearrange("(q t) f -> q t f", q=P)
                )
                # fp32r-rounded copy of x for the hT-accumulation stationaries
                x_sbr = xpool.tile([P, TBLK * F], F32R, tag="x_sbr")
                eng.copy(x_sbr[:], x_sb[:].rearrange("q t f -> q (t f)"))

                # ---- xT (augmented with ones row): column t*128+q <-> row 16q+t
                for c in range(NCH):
                    pt = ps_small.tile([F, CH], F32, tag="pss")
                    for j in range(CH // P):
                        t = c * (CH // P) + j
                        nc.tensor.transpose(pt[:, ts(j, P)], x_sb[:, t, :], ident[:])
                    eng.copy(xta[0:F, ts(c, CH)], pt[:])

                # ---- e1T / e2T = Wa^T @ xTa   (biases included via K=65)
                ets = []
                for wa, tag in ((w1a, "e1t"), (w2a, "e2t")):
                    et = epool.tile([H, N], F32R, tag=tag)
                    for c in range(NCH):
                        pe_ = ps_small.tile([H, CH], F32, tag="pss")
                        nc.tensor.matmul(
                            pe_[:],
                            wa[:],
                            xta[:, ts(c, CH)],
                            start=True,
                            stop=True,
                        )
                        eng.copy(et[:, ts(c, CH)], pe_[:])
                    ets.append(et)
                e1t, e2t = ets

                # ---- main fused loop over 16 m-blocks:
                #      adjT_mb = relu(e2t[:, mb]^T @ e1t)      [128, 2048]
                #      hT     += x_mb(stationary) @ adjT_mb    [64, 2048] PSUM
                ph = [
                    ps_h.tile([F, CH], F32, tag="ph", name=f"ph{c}")
                    for c in range(NCH)
                ]
                for mb in range(TBLK):
                    for c in range(NCH):
                        pa = ps_adj.tile([P, CH], F32, tag="pa")
                        nc.tensor.matmul(
                            pa[:],
                            e2t[:, ts(mb, P)],
                            e1t[:, ts(c, CH)],
                            start=True,
                            stop=True,
                        )
                        asb = adjpool.tile([P, CH], F32R, tag="asb")
                        eng.relu(asb[:], pa[:])
                        nc.tensor.matmul(
                            ph[c][:],
                            x_sbr[:, ts(mb, F)],
                            asb[:],
                            start=(mb == 0),
                            stop=(mb == TBLK - 1),
                        )

                # ---- hT -> SBUF (augmented with ones row for the b3 fold)
                hta = htpool.tile([F + 1, N], F32, tag="hta")
                nc.gpsimd.memset(hta[F : F + 1, :], 1.0)
                for c in range(NCH):
                    eng.copy(hta[0:F, ts(c, CH)], ph[c][:])

                # ---- out = hTa^T @ W3a  (b3 via K=65), packed 8 blocks/bank
                out_sb = opool.tile([P, TBLK * O], F32, tag="out_sb")
                for g in range(2):
                    po = ps_small.tile([P, CH], F32, tag="pss")
                    for j in range(TBLK // 2):
                        nb = g * (TBLK // 2) + j
                        nc.tensor.matmul(
                            po[:, ts(j, O)],
                            hta[:, ts(nb, P)],
                            w3a[:],
                            start=True,
                            stop=True,
                        )
                    eng.copy(out_sb[:, ts(g, CH)], po[:])
                nc.sync.dma_start(
                    out_d[p].rearrange("(q t) f -> q (t f)", q=P), out_sb[:]
                )

        if reps == 1:
            body()
        else:
            with tc.For_i(0, reps, 1):
                body()


def build_program(reps=1):
    nc = bacc.Bacc("TRN2", target_bir_lowering=False, debug=False)
    x_d = nc.dram_tensor("x", [PAIRS, N, F], F32, kind="ExternalInput").ap()
    w_d = [
        nc.dram_tensor(f"w{k}", [H, F], F32, kind="ExternalInput").ap()
        for k in (1, 2, 3)
    ]
    b_d = [
        nc.dram_tensor(f"b{k}", [H], F32, kind="ExternalInput").ap()
        for k in (1, 2, 3)
    ]
    out_d = nc.dram_tensor("out", [PAIRS, N, O], F32, kind="ExternalOutput").ap()
    with tile.TileContext(nc) as tc:
        _emit(tc, x_d, w_d, b_d, out_d, reps=reps)
    nc.compile()
    return nc


def make_in_maps(x, W1, b1, W2, b2, W3, b3):
    xs = np.ascontiguousarray(np.asarray(x, np.float32).reshape(B * C, N, F))
    const = {
        "w1": np.ascontiguousarray(np.asarray(W1, np.float32)),
        "w2": np.ascontiguousarray(np.asarray(W2, np.float32)),
        "w3": np.ascontiguousarray(np.asarray(W3, np.float32)),
        "b1": np.ascontiguousarray(np.asarray(b1, np.float32)),
        "b2": np.ascontiguousarray(np.asarray(b2, np.float32)),
        "b3": np.ascontiguousarray(np.asarray(b3, np.float32)),
    }
    return [
        {"x": np.ascontiguousarray(xs[i * PAIRS : (i + 1) * PAIRS]), **const}
        for i in range(NCORES)
    ]


_NC_CACHE = {}


def kernel(x, W1, b1, W2, b2, W3, b3):
    from concourse.bass_utils import run_bass_kernel_spmd

    if "nc" not in _NC_CACHE:
        _NC_CACHE["nc"] = build_program()
    nc = _NC_CACHE["nc"]
    in_maps = make_in_maps(x, W1, b1, W2, b2, W3, b3)
    res = run_bass_kernel_spmd(nc, in_maps, list(range(NCORES))).results
    out = np.concatenate([res[i]["out"] for i in range(NCORES)], axis=0)
    return out.reshape(B, C, N, O)


# revision 14
# speedup vs baseline: 11.2402x; 1.0785x over previous
"""GCN message-passing kernel for 8 TRN2 NeuronCores.

Reference computation (per (b, c) pair, all fp32):
    e1  = x @ W1^T + b1          [N, H]
    e2  = x @ W2^T + b2          [N, H]
    adj = relu(e1 @ e2^T)        [N, N]
    h   = adj @ x                [N, F]
    out = h @ W3^T + b3          [N, O]

Sharding: the 32 (b, c) pairs are split 4-per-core across 8 cores;
weights are replicated. Each core runs an identical Bass program fully
fused in SBUF/PSUM (the N x N adjacency never touches HBM).

Layout trick: partition p of SBUF holds rows [16p, 16p+16) of the pair
(a pure row permutation, applied consistently to both sides of every
contraction and undone by the output DMA), which makes every HBM
transfer contiguous 4KB per partition.
"""

import sys

for _p in ("/opt/trn_rl_repo",):
    if _p not in sys.path:
        sys.path.insert(0, _p)

import numpy as np

import concourse.bass as bass
import concourse.tile as tile
from concourse import bacc, mybir
from concourse.bass import ts
from concourse.masks import make_identity

B, C, N, F = 4, 8, 2048, 64
H = 64
O = 64
NCORES = 8
PAIRS = (B * C) // NCORES  # 4 (b,c) pairs per core
P = 128                    # SBUF partitions
TBLK = N // P              # 16 row-blocks per pair
CH = 512                   # moving-operand chunk (one PSUM bank of fp32)
NCH = N // CH              # 4 chunks per row
F32 = mybir.dt.float32
F32R = mybir.dt.float32r

AF = mybir.ActivationFunctionType


class _EngineAlternator:
    """Round-robin PSUM->SBUF copy/relu work across Scalar and Vector."""

    def __init__(self, nc):
        self.nc = nc
        self.i = 0

    def copy(self, out, in_):
        self.i += 1
        if self.i % 2:
            self.nc.scalar.copy(out, in_)
        else:
            self.nc.vector.tensor_copy(out, in_)

    def relu(self, out, in_):
        self.i += 1
        if self.i % 2:
            self.nc.scalar.activation(out, in_, AF.Relu)
        else:
            self.nc.vector.tensor_scalar_max(out, in_, 0.0)


def _emit(tc, x_d, w_d, b_d, out_d, reps=1):
    nc = tc.nc
    eng = _EngineAlternator(nc)

    import contextlib

    with contextlib.ExitStack() as ctx:
        consts = ctx.enter_context(tc.tile_pool(name="consts", bufs=1))
        xpool = ctx.enter_context(tc.tile_pool(name="xp", bufs=2))
        xtpool = ctx.enter_context(tc.tile_pool(name="xt", bufs=2))
        epool = ctx.enter_context(tc.tile_pool(name="ep", bufs=2))
        adjpool = ctx.enter_context(tc.tile_pool(name="adj", bufs=4))
        htpool = ctx.enter_context(tc.tile_pool(name="ht", bufs=2))
        opool = ctx.enter_context(tc.tile_pool(name="op", bufs=2))
        ps_small = ctx.enter_context(tc.tile_pool(name="pss", bufs=2, space="PSUM"))
        ps_adj = ctx.enter_context(tc.tile_pool(name="psa", bufs=2, space="PSUM"))
        ps_h = ctx.enter_context(tc.tile_pool(name="psh", bufs=4, space="PSUM"))

        ident = consts.tile([P, P], F32)
        make_identity(nc, ident[:])

        # Augmented transposed weights: wa[k] rows 0..F-1 = Wk^T, row F = bk,
        # so e = Wa^T @ [xT; ones] folds the bias into the matmul (K = F+1).
        # w1a/w2a feed fp32r matmuls, so every producer writes fp32r
        # (the BIR verifier requires fp32r operands to be pre-rounded).
        was = []
        for k in range(3):
            dt_k = F32 if k == 2 else F32R
            wraw = consts.tile([H, F], F32, tag="wraw")
            nc.sync.dma_start(wraw[:], w_d[k][:])
            brow = consts.tile([1, H], F32, tag="brow")
            nc.sync.dma_start(brow[:], b_d[k].unsqueeze(0))
            wa = consts.tile([F + 1, H], dt_k, tag=f"wa{k}")
            pw = ps_small.tile([F, H], F32, tag="pss")
            nc.tensor.transpose(pw[:], wraw[:], ident[0:H, 0:H])
            nc.vector.tensor_copy(wa[0:F, :], pw[:])
            nc.vector.tensor_copy(wa[F : F + 1, :], brow[:])
            was.append(wa)
        w1a, w2a, w3a = was

        # Persistent xT tile (fp32r): Pool memset can't write fp32r, so the
        # ones row is seeded once from an fp32 staging row; rows 0..F-1 are
        # overwritten per pair.
        ones_row = consts.tile([1, N], F32)
        nc.gpsimd.memset(ones_row[:], 1.0)
        xta = consts.tile([F + 1, N], F32R)
        nc.vector.tensor_copy(xta[F : F + 1, :], ones_row[:])

        def prep(p):
            """Pair prologue: x load, fp32r round, xT transposes, e1T/e2T.
            Emitted mid-way through the previous pair's main loop so the PE
            queue never drains at a pair boundary."""
            x_sb = xpool.tile([P, TBLK, F], F32, tag="x_sb", name=f"x_sb{p}")
            nc.sync.dma_start(
                x_sb[:], x_d[p].rearrange("(q t) f -> q t f", q=P)
            )
            # fp32r-rounded copy of x for the hT-accumulation stationaries
            x_sbr = xpool.tile([P, TBLK * F], F32R, tag="x_sbr", name=f"x_sbr{p}")
            eng.copy(x_sbr[:], x_sb[:].rearrange("q t f -> q (t f)"))

            # xT (augmented with ones row): column t*128+q <-> row 16q+t
            for c in range(NCH):
                pt = ps_small.tile([F, CH], F32, tag="pss", name=f"pt{p}_{c}")
                for j in range(CH // P):
                    t = c * (CH // P) + j
                    nc.tensor.transpose(pt[:, ts(j, P)], x_sb[:, t, :], ident[:])
                eng.copy(xta[0:F, ts(c, CH)], pt[:])

            # e1T / e2T = Wa^T @ xTa   (biases included via K=65)
            ets = []
            for wa, tag in ((w1a, "e1t"), (w2a, "e2t")):
                et = epool.tile([H, N], F32R, tag=tag, name=f"{tag}{p}")
                for c in range(NCH):
                    pe_ = ps_small.tile([H, CH], F32, tag="pss", name=f"pe{p}_{c}")
                    nc.tensor.matmul(
                        pe_[:], wa[:], xta[:, ts(c, CH)], start=True, stop=True
                    )
                    eng.copy(et[:, ts(c, CH)], pe_[:])
                ets.append(et)
            return x_sb, x_sbr, ets[0], ets[1]

        def main(p, st, tail_emit):
            """Main fused loop for pair p. Interleaves the deferred tail of
            pair p-1 (early) and the prologue of pair p+1 (late)."""
            x_sb, x_sbr, e1t, e2t = st
            ph = [
                ps_h.tile([F, CH], F32, tag="ph", name=f"ph{p}_{c}")
                for c in range(NCH)
            ]
            next_st = None
            for mb in range(TBLK):
                for c in range(NCH):
                    pa = ps_adj.tile([P, CH], F32, tag="pa", name=f"pa{p}_{mb}_{c}")
                    nc.tensor.matmul(
                        pa[:], e2t[:, ts(mb, P)], e1t[:, ts(c, CH)],
                        start=True, stop=True,
                    )
                    asb = adjpool.tile([P, CH], F32R, tag="asb",
                                       name=f"asb{p}_{mb}_{c}")
                    eng.relu(asb[:], pa[:])
                    nc.tensor.matmul(
                        ph[c][:], x_sbr[:, ts(mb, F)], asb[:],
                        start=(mb == 0), stop=(mb == TBLK - 1),
                    )
                if mb == 3 and tail_emit is not None:
                    tail_emit()
                    tail_emit = None
                if mb == 8 and p + 1 < PAIRS:
                    next_st = prep(p + 1)

            # hT -> SBUF right away (frees the 4 ph PSUM banks; DVE/ACT work
            # that overlaps the next pair's PE stream)
            hta = htpool.tile([F + 1, N], F32, tag="hta", name=f"hta{p}")
            nc.gpsimd.memset(hta[F : F + 1, :], 1.0)
            for c in range(NCH):
                eng.copy(hta[0:F, ts(c, CH)], ph[c][:])

            def tail():
                # out = hTa^T @ W3a  (b3 via K=65), packed 8 blocks/bank
                out_sb = opool.tile([P, TBLK * O], F32, tag="out_sb",
                                    name=f"out_sb{p}")
                for g in range(2):
                    po = ps_small.tile([P, CH], F32, tag="pss", name=f"po{p}_{g}")
                    for j in range(TBLK // 2):
                        nb = g * (TBLK // 2) + j
                        nc.tensor.matmul(
                            po[:, ts(j, O)], hta[:, ts(nb, P)], w3a[:],
                            start=True, stop=True,
                        )
                    eng.copy(out_sb[:, ts(g, CH)], po[:])
                nc.sync.dma_start(
                    out_d[p].rearrange("(q t) f -> q (t f)", q=P), out_sb[:]
                )

            return next_st, tail

        def body():
            st = prep(0)
            tail = None
            for p in range(PAIRS):
                st, tail = main(p, st, tail)
            tail()

        if reps == 1:
            body()
        else:
            with tc.For_i(0, reps, 1):
                body()


def build_program(reps=1):
    nc = bacc.Bacc("TRN2", target_bir_lowering=False, debug=False)
    x_d = nc.dram_tensor("x", [PAIRS, N, F], F32, kind="ExternalInput").ap()
    w_d = [
        nc.dram_tensor(f"w{k}", [H, F], F32, kind="ExternalInput").ap()
        for k in (1, 2, 3)
    ]
    b_d = [
        nc.dram_tensor(f"b{k}", [H], F32, kind="ExternalInput").ap()
        for k in (1, 2, 3)
    ]
    out_d = nc.dram_tensor("out", [PAIRS, N, O], F32, kind="ExternalOutput").ap()
    with tile.TileContext(nc) as tc:
        _emit(tc, x_d, w_d, b_d, out_d, reps=reps)
    nc.compile()
    return nc


def make_in_maps(x, W1, b1, W2, b2, W3, b3):
    xs = np.ascontiguousarray(np.asarray(x, np.float32).reshape(B * C, N, F))
    const = {
        "w1": np.ascontiguousarray(np.asarray(W1, np.float32)),
        "w2": np.ascontiguousarray(np.asarray(W2, np.float32)),
        "w3": np.ascontiguousarray(np.asarray(W3, np.float32)),
        "b1": np.ascontiguousarray(np.asarray(b1, np.float32)),
        "b2": np.ascontiguousarray(np.asarray(b2, np.float32)),
        "b3": np.ascontiguousarray(np.asarray(b3, np.float32)),
    }
    return [
        {"x": np.ascontiguousarray(xs[i * PAIRS : (i + 1) * PAIRS]), **const}
        for i in range(NCORES)
    ]


_NC_CACHE = {}


def kernel(x, W1, b1, W2, b2, W3, b3):
    from concourse.bass_utils import run_bass_kernel_spmd

    if "nc" not in _NC_CACHE:
        _NC_CACHE["nc"] = build_program()
    nc = _NC_CACHE["nc"]
    in_maps = make_in_maps(x, W1, b1, W2, b2, W3, b3)
    res = run_bass_kernel_spmd(nc, in_maps, list(range(NCORES))).results
    out = np.concatenate([res[i]["out"] for i in range(NCORES)], axis=0)
    return out.reshape(B, C, N, O)
